# revision 5
# baseline (speedup 1.0000x reference)
"""Trainium2 Bass kernel for nn_Expert_13082470383822.

y = silu(depthwise_causal_conv1d(x, conv_w, K=4) + conv_b);  out = y @ W_proj.T + b_proj
x [4, 4096, 2048] fp32. Data-parallel over the 16384 (batch*seq) tokens across
8 NeuronCores (2048 tokens/core + 3-token halo).

All-bf16 dataflow (x, W, y, out in bf16; PSUM accum fp32). DMA engines are
element-rate limited, so every large transfer is packed as fp32 pairs in DRAM
and bitcast to bf16 in SBUF (2x effective DMA bandwidth vs raw bf16).

Conv runs on 512-token chunks: tap 0 on ACT (copy with per-partition scale),
taps 1-3 as DVE scalar_tensor_tensor chains, SiLU+conv_b on ACT writing bf16 y.
Projection on the PE: y tiles stationary [128ch x 128tok] (bf16 LDWEIGHTS are
cheap and hide under matmuls via the PE reorder window), W moving
[128ch x 512feat], fp32 PSUM accumulation over the 16 channel tiles.

W streams across all three DMA queues (SP: j0-8, ACT-HWDGE: j9-13 after the
first x chunk, gpsimd-SWDGE: j14-15) so its arrival curve stays ahead of the
PE's in-order consumption during chunk 0, which interleaves two token strips
per channel tile (8 matmuls per W tile) to chase the stream. Later chunks go
strip-by-strip with 4-bank PSUM ping-pong. Conv for chunk c+1 is emitted before
the PE work of chunk c so the DVE FIFO never blocks the next conv behind PSUM
drains. b_proj is added during the PSUM->SBUF drain on DVE; output rows stream
out bf16 (fp32-packed) on the SP queue and are unpacked on host.
"""

import sys

if "/opt/trn_rl_repo" not in sys.path:
    sys.path.insert(0, "/opt/trn_rl_repo")

import numpy as np
import ml_dtypes

B, S, D, KW = 4, 4096, 2048, 4
NCORES = 8
T = (B * S) // NCORES  # tokens per core = 2048
KT = D // 128  # 16 channel tiles
ECH = D // 512  # 4 e-chunks
CW = 512  # conv chunk width (tokens)
MS = 128  # matmul strip width (tokens)
NCH = T // CW  # 4 conv chunks
SPC = CW // MS  # 4 matmul strips per chunk
JP = 4  # channel tiles per x DMA piece
NPC = KT // JP  # 4 x pieces per chunk
XW = JP * (CW + 3)  # flat bf16 elements per x piece row (2060)

_BUILT = {}


def _build_program():
    if "nc" in _BUILT:
        return _BUILT["nc"]

    import concourse.tile as tile
    from concourse import bacc, mybir

    dt = mybir.dt
    AF = mybir.ActivationFunctionType
    ALU = mybir.AluOpType

    nc = bacc.Bacc("TRN2", target_bir_lowering=False, debug=False)
    # all bulk tensors are packed as fp32 pairs of bf16 for full DMA rate
    xs_d = nc.declare_dram_parameter(
        "xs_t", [NCH * NPC, 128, XW // 2], dt.float32, isOutput=False
    )
    wt = nc.declare_dram_parameter("wt", [D, D // 2], dt.float32, isOutput=False)
    cw = nc.declare_dram_parameter("cw", [128, KT * KW], dt.float32, isOutput=False)
    cb = nc.declare_dram_parameter("cb", [128, KT], dt.float32, isOutput=False)
    bp = nc.declare_dram_parameter("bp", [1, D], dt.float32, isOutput=False)
    out = nc.declare_dram_parameter("out", [T, D // 2], dt.float32, isOutput=True)

    with tile.TileContext(nc) as tc:
        with (
            tc.tile_pool(name="consts", bufs=1) as cpool,
            tc.tile_pool(name="wpool", bufs=1) as wpool,
            tc.tile_pool(name="xpool", bufs=6) as xpool,
            tc.tile_pool(name="ypool", bufs=3) as ypool,
            tc.tile_pool(name="apool", bufs=4) as apool,
            tc.tile_pool(name="opool", bufs=3) as opool,
            tc.tile_pool(name="pspool", bufs=8, space="PSUM") as pspool,
        ):
            # warm the ACT function table before any real work
            dum = cpool.tile([1, 1], dt.float32, name="dum")
            nc.gpsimd.memset(dum[:, :], 0.0)
            nc.scalar.activation(dum[:, :], dum[:, :], AF.Silu, bias=0.0)

            cw_sb = cpool.tile([128, KT * KW], dt.float32, name="cw_sb")
            nc.gpsimd.dma_start(out=cw_sb[:, :], in_=cw[:, :])
            cb_sb = cpool.tile([128, KT], dt.float32, name="cb_sb")
            nc.gpsimd.dma_start(out=cb_sb[:, :], in_=cb[:, :])

            w_sb = [
                wpool.tile([128, D], dt.bfloat16, name=f"w{j}") for j in range(KT)
            ]

            def w_dma(eng, j):
                eng.dma_start(
                    out=w_sb[j].bitcast(dt.float32),
                    in_=wt[j * 128 : (j + 1) * 128, :],
                )

            # W j0-8 on the SP queue from t=0
            for j in range(9):
                w_dma(nc.sync, j)
            # W j14-15 + bias broadcast ride the gpsimd SWDGE behind cw/cb
            w_dma(nc.gpsimd, 14)
            w_dma(nc.gpsimd, 15)
            bb_sb = cpool.tile([128, D], dt.float32, name="bb_sb")
            nc.gpsimd.dma_start(out=bb_sb[:, :], in_=bp[:, :].broadcast_to([128, D]))

            def emit_conv(c):
                pieces = []
                for p in range(NPC):
                    xt = xpool.tile([128, XW], dt.bfloat16, name="xs", tag="xs")
                    # chunk 0 loads on the ACT HWDGE queue (fast, needed first);
                    # later chunks trickle in on the gpsimd SWDGE
                    eng = nc.scalar if c == 0 else nc.gpsimd
                    eng.dma_start(
                        out=xt.bitcast(dt.float32), in_=xs_d[NPC * c + p, :, :]
                    )
                    pieces.append(xt)

                y = ypool.tile([128, KT, CW], dt.bfloat16, name="ys", tag="ys")
                accs = [None] * KT
                accs[0] = apool.tile([128, CW], dt.bfloat16, name="acc", tag="acc")
                nc.scalar.activation(
                    accs[0][:, :],
                    pieces[0][:, 0:CW],
                    AF.Copy,
                    bias=0.0,
                    scale=cw_sb[:, 0:1],
                )
                for j in range(KT):
                    xs = pieces[j // JP]
                    base = (j % JP) * (CW + 3)
                    for k in range(1, KW):
                        nc.vector.scalar_tensor_tensor(
                            accs[j][:, :],
                            xs[:, base + k : base + k + CW],
                            cw_sb[:, j * KW + k : j * KW + k + 1],
                            accs[j][:, :],
                            ALU.mult,
                            ALU.add,
                        )
                    # tap 0 of the NEXT tile before silu(j): keeps ACT ahead of
                    # the DVE chain instead of FIFO-stalling behind it
                    if j + 1 < KT:
                        accs[j + 1] = apool.tile(
                            [128, CW], dt.bfloat16, name="acc", tag="acc"
                        )
                        nxt = pieces[(j + 1) // JP]
                        nb = ((j + 1) % JP) * (CW + 3)
                        nc.scalar.activation(
                            accs[j + 1][:, :],
                            nxt[:, nb : nb + CW],
                            AF.Copy,
                            bias=0.0,
                            scale=cw_sb[:, (j + 1) * KW : (j + 1) * KW + 1],
                        )
                    nc.scalar.activation(
                        y[:, j, :],
                        accs[j][:, :],
                        AF.Silu,
                        bias=cb_sb[:, j : j + 1],
                    )
                    if c == 0 and j == 0:
                        # W j9-13 on the ACT queue, issued after chunk-0 x and
                        # the first conv ops so they don't delay either
                        for jj in range(9, 14):
                            w_dma(nc.scalar, jj)
                return y

            def emit_pe(c, y):
                # chunk 0 interleaves strip pairs per channel tile so the PE
                # keeps pace with the streaming W tiles; later chunks go
                # strip-by-strip with 4-bank PSUM ping-pong
                groups = [(0, 1), (2, 3)] if c == 0 else [(0,), (1,), (2,), (3,)]
                for group in groups:
                    pss = {}
                    for m in group:
                        pss[m] = [
                            pspool.tile([128, 512], dt.float32, name="ps", tag="ps")
                            for _ in range(ECH)
                        ]
                    for j in range(KT):
                        for m in group:
                            for e in range(ECH):
                                nc.tensor.matmul(
                                    pss[m][e][:, :],
                                    y[:, j, m * MS : (m + 1) * MS],
                                    w_sb[j][:, e * 512 : (e + 1) * 512],
                                    start=(j == 0),
                                    stop=(j == KT - 1),
                                )
                    for m in group:
                        os_sb = opool.tile([128, D], dt.bfloat16, name="os", tag="os")
                        for e in range(ECH):
                            nc.vector.tensor_tensor(
                                out=os_sb[:, e * 512 : (e + 1) * 512],
                                in0=pss[m][e][:, :],
                                in1=bb_sb[:, e * 512 : (e + 1) * 512],
                                op=ALU.add,
                            )
                        s = c * SPC + m
                        nc.sync.dma_start(
                            out=out[s * MS : (s + 1) * MS, :],
                            in_=os_sb.bitcast(dt.float32),
                        )

            ys = []
            for c in range(NCH):
                ys.append(emit_conv(c))
                if c >= 1:
                    emit_pe(c - 1, ys[c - 1])
            emit_pe(NCH - 1, ys[NCH - 1])

    nc.compile()
    _BUILT["nc"] = nc
    return nc


def _shard_inputs(x, conv_w, conv_b, W_proj, b_proj):
    bf16 = ml_dtypes.bfloat16
    x = np.ascontiguousarray(x, dtype=np.float32)
    wt_np = (
        np.ascontiguousarray(W_proj.T, dtype=np.float32)
        .astype(bf16)
        .view(np.float32)
    )
    cw_np = np.ascontiguousarray(
        conv_w.reshape(KT, 128, KW).transpose(1, 0, 2).reshape(128, KT * KW),
        dtype=np.float32,
    )
    cb_np = np.ascontiguousarray(conv_b.reshape(KT, 128).T, dtype=np.float32)
    bp_np = np.ascontiguousarray(b_proj.reshape(1, D), dtype=np.float32)

    per_batch = S // T
    in_maps = []
    for c in range(NCORES):
        b = c // per_batch
        s0 = (c % per_batch) * T
        xp = np.zeros((T + 3, D), dtype=np.float32)
        xp[3:] = x[b, s0 : s0 + T]
        if s0 > 0:
            xp[:3] = x[b, s0 - 3 : s0]
        xTc = xp.T  # [D, T+3]
        # [NCH, D, CW+3] sliding chunks -> [NCH, NPC, JP, 128, CW+3]
        # -> [NCH*NPC, 128, JP*(CW+3)] bf16, fp32-packed for DMA
        chunks = np.stack([xTc[:, i * CW : i * CW + CW + 3] for i in range(NCH)])
        chunks = chunks.reshape(NCH, NPC, JP, 128, CW + 3)
        pieces = np.ascontiguousarray(chunks.transpose(0, 1, 3, 2, 4)).reshape(
            NCH * NPC, 128, XW
        )
        in_maps.append(
            {
                "xs_t": pieces.astype(bf16).view(np.float32),
                "wt": wt_np,
                "cw": cw_np,
                "cb": cb_np,
                "bp": bp_np,
            }
        )
    return in_maps


def run_sharded(x, conv_w, conv_b, W_proj, b_proj, trace=False):
    """Run across the 8 cores; returns (full_out [B,S,D], BassKernelResults)."""
    from concourse.bass_utils import run_bass_kernel_spmd

    nc = _build_program()
    in_maps = _shard_inputs(x, conv_w, conv_b, W_proj, b_proj)
    try:
        res = run_bass_kernel_spmd(nc, in_maps, list(range(NCORES)), trace=trace)
    except Exception:
        # transient device wedges (NRT_EXEC_UNIT_UNRECOVERABLE) clear on retry
        res = run_bass_kernel_spmd(nc, in_maps, list(range(NCORES)), trace=trace)
    full = np.empty((B, S, D), dtype=np.float32)
    per_batch = S // T
    for c in range(NCORES):
        b = c // per_batch
        s0 = (c % per_batch) * T
        o = np.ascontiguousarray(res.results[c]["out"])
        full[b, s0 : s0 + T] = o.view(ml_dtypes.bfloat16).astype(np.float32)
    return full, res


def kernel(x, conv_w, conv_b, W_proj, b_proj):
    full, _ = run_sharded(x, conv_w, conv_b, W_proj, b_proj, trace=False)
    return full


# revision 7
# speedup vs baseline: 1.0313x; 1.0313x over previous
"""Trainium2 Bass kernel for nn_Expert_13082470383822.

y = silu(depthwise_causal_conv1d(x, conv_w, K=4) + conv_b);  out = y @ W_proj.T + b_proj
x [4, 4096, 2048] fp32. Data-parallel over the 16384 (batch*seq) tokens across
8 NeuronCores (2048 tokens/core + 3-token halo).

All-bf16 dataflow (x, W, y, out in bf16; PSUM accum fp32). DMA queues are
serial with ~5us turnaround per 128-row transfer, so every bulk tensor moves
as few, fat, fp32-packed transfers: x in 1MB chunk-halves, W in 1MB
channel-tile pairs, out in 1MB strip pairs; bf16 payloads are bitcast to fp32
rows so the element-rate-limited engines see half the elements.

Queue discipline (waits on a DMA issue stall the issuing engine, so compute
engines carry only wait-free issues): chunk-0 x rides the ACT HWDGE queue as
its only two DMAs, issued ahead of all ACT compute; W pairs 0-2 go on the SP
queue followed by the output pairs; W pairs 3-7 plus the bias broadcast and
the remaining x chunks trickle through the gpsimd SWDGE.

Conv runs on 512-token chunks: tap 0 on ACT (copy with per-partition scale),
taps 1-3 as DVE scalar_tensor_tensor chains, SiLU+conv_b on ACT writing bf16 y.
Projection on the PE: y tiles stationary [128ch x 128tok] (bf16 LDWEIGHTS hide
under matmuls via the PE reorder window), W moving [128ch x 512feat], fp32 PSUM
accumulation over the 16 channel tiles. Chunk 0 interleaves two token strips
per channel tile (8 matmuls per W tile) to chase the W stream; later chunks go
strip-by-strip with 4-bank PSUM ping-pong. Conv for chunk c+1 is emitted before
the PE work of chunk c so the DVE FIFO never blocks the next conv behind PSUM
drains. b_proj is added during the PSUM->SBUF drain on DVE.
"""

import sys

if "/opt/trn_rl_repo" not in sys.path:
    sys.path.insert(0, "/opt/trn_rl_repo")

import numpy as np
import ml_dtypes

B, S, D, KW = 4, 4096, 2048, 4
NCORES = 8
T = (B * S) // NCORES  # tokens per core = 2048
KT = D // 128  # 16 channel tiles
ECH = D // 512  # 4 e-chunks
CW = 512  # conv chunk width (tokens)
MS = 128  # matmul strip width (tokens)
NCH = T // CW  # 4 conv chunks
SPC = CW // MS  # 4 matmul strips per chunk
JH = KT // 2  # 8 channel tiles per x half
XW = JH * (CW + 3)  # flat bf16 elements per x half row (4120)
NWP = KT // 2  # 8 W pair tiles
NOP = T // MS // 2  # 8 out strip pairs

_BUILT = {}


def _build_program():
    if "nc" in _BUILT:
        return _BUILT["nc"]

    import concourse.tile as tile
    from concourse import bacc, mybir

    dt = mybir.dt
    AF = mybir.ActivationFunctionType
    ALU = mybir.AluOpType

    nc = bacc.Bacc("TRN2", target_bir_lowering=False, debug=False)
    # all bulk tensors are packed as fp32 pairs of bf16 for full DMA rate
    xs_d = nc.declare_dram_parameter(
        "xs_t", [NCH * 2, 128, XW // 2], dt.float32, isOutput=False
    )
    wt = nc.declare_dram_parameter(
        "wt", [NWP, 128, 2, D // 2], dt.float32, isOutput=False
    )
    cw = nc.declare_dram_parameter("cw", [128, KT * KW], dt.float32, isOutput=False)
    cb = nc.declare_dram_parameter("cb", [128, KT], dt.float32, isOutput=False)
    bp = nc.declare_dram_parameter("bp", [1, D], dt.float32, isOutput=False)
    out = nc.declare_dram_parameter(
        "out", [NOP, 128, 2, D // 2], dt.float32, isOutput=True
    )

    with tile.TileContext(nc) as tc:
        with (
            tc.tile_pool(name="consts", bufs=1) as cpool,
            tc.tile_pool(name="wpool", bufs=1) as wpool,
            tc.tile_pool(name="xpool", bufs=4) as xpool,
            tc.tile_pool(name="ypool", bufs=3) as ypool,
            tc.tile_pool(name="apool", bufs=4) as apool,
            tc.tile_pool(name="opool", bufs=2) as opool,
            tc.tile_pool(name="pspool", bufs=8, space="PSUM") as pspool,
        ):
            # chunk-0 x halves: the ACT queue's only DMAs, issued before any
            # ACT compute so the issues never wait on semaphore reuse
            x0 = []
            for h in range(2):
                xt = xpool.tile([128, XW], dt.bfloat16, name="xs", tag="xs")
                nc.scalar.dma_start(out=xt.bitcast(dt.float32), in_=xs_d[h, :, :])
                x0.append(xt)

            # warm the ACT function table
            dum = cpool.tile([1, 1], dt.float32, name="dum")
            nc.gpsimd.memset(dum[:, :], 0.0)
            nc.scalar.activation(dum[:, :], dum[:, :], AF.Silu, bias=0.0)

            cw_sb = cpool.tile([128, KT * KW], dt.float32, name="cw_sb")
            nc.gpsimd.dma_start(out=cw_sb[:, :], in_=cw[:, :])
            cb_sb = cpool.tile([128, KT], dt.float32, name="cb_sb")
            nc.gpsimd.dma_start(out=cb_sb[:, :], in_=cb[:, :])

            # W pairs: first 3 on SP (feeds the in-order chase), rest + bias
            # broadcast on the gpsimd SWDGE
            w_sb = [
                wpool.tile([128, 2, D], dt.bfloat16, name=f"w{p}") for p in range(NWP)
            ]
            for p in range(NWP):
                eng = nc.sync if p < 3 else nc.gpsimd
                eng.dma_start(out=w_sb[p].bitcast(dt.float32), in_=wt[p, :, :, :])
            bb_sb = cpool.tile([128, D], dt.float32, name="bb_sb")
            nc.gpsimd.dma_start(out=bb_sb[:, :], in_=bp[:, :].broadcast_to([128, D]))

            def w_ap(j, e):
                return w_sb[j // 2][:, j % 2, e * 512 : (e + 1) * 512]

            def emit_conv(c):
                if c == 0:
                    pieces = x0
                else:
                    pieces = []
                    for h in range(2):
                        xt = xpool.tile(
                            [128, XW], dt.bfloat16, name="xs", tag="xs"
                        )
                        nc.gpsimd.dma_start(
                            out=xt.bitcast(dt.float32), in_=xs_d[2 * c + h, :, :]
                        )
                        pieces.append(xt)

                y = ypool.tile([128, KT, CW], dt.bfloat16, name="ys", tag="ys")
                accs = [None] * KT
                accs[0] = apool.tile([128, CW], dt.bfloat16, name="acc", tag="acc")
                nc.scalar.activation(
                    accs[0][:, :],
                    pieces[0][:, 0:CW],
                    AF.Copy,
                    bias=0.0,
                    scale=cw_sb[:, 0:1],
                )
                for j in range(KT):
                    xs = pieces[j // JH]
                    base = (j % JH) * (CW + 3)
                    for k in range(1, KW):
                        nc.vector.scalar_tensor_tensor(
                            accs[j][:, :],
                            xs[:, base + k : base + k + CW],
                            cw_sb[:, j * KW + k : j * KW + k + 1],
                            accs[j][:, :],
                            ALU.mult,
                            ALU.add,
                        )
                    # tap 0 of the NEXT tile before silu(j): keeps ACT ahead of
                    # the DVE chain instead of FIFO-stalling behind it
                    if j + 1 < KT:
                        accs[j + 1] = apool.tile(
                            [128, CW], dt.bfloat16, name="acc", tag="acc"
                        )
                        nxt = pieces[(j + 1) // JH]
                        nb = ((j + 1) % JH) * (CW + 3)
                        nc.scalar.activation(
                            accs[j + 1][:, :],
                            nxt[:, nb : nb + CW],
                            AF.Copy,
                            bias=0.0,
                            scale=cw_sb[:, (j + 1) * KW : (j + 1) * KW + 1],
                        )
                    nc.scalar.activation(
                        y[:, j, :],
                        accs[j][:, :],
                        AF.Silu,
                        bias=cb_sb[:, j : j + 1],
                    )
                return y

            os_cur = [None]  # current out strip-pair tile

            def emit_pe(c, y):
                # chunk 0 interleaves strip pairs per channel tile so the PE
                # keeps pace with the streaming W pairs; later chunks go
                # strip-by-strip with 4-bank PSUM ping-pong
                groups = [(0, 1), (2, 3)] if c == 0 else [(0,), (1,), (2,), (3,)]
                for group in groups:
                    pss = {}
                    for m in group:
                        pss[m] = [
                            pspool.tile([128, 512], dt.float32, name="ps", tag="ps")
                            for _ in range(ECH)
                        ]
                    for j in range(KT):
                        for m in group:
                            for e in range(ECH):
                                nc.tensor.matmul(
                                    pss[m][e][:, :],
                                    y[:, j, m * MS : (m + 1) * MS],
                                    w_ap(j, e),
                                    start=(j == 0),
                                    stop=(j == KT - 1),
                                )
                    for m in group:
                        s = c * SPC + m
                        if s % 2 == 0:
                            os_cur[0] = opool.tile(
                                [128, 2, D], dt.bfloat16, name="os", tag="os"
                            )
                        os_sb = os_cur[0]
                        for e in range(ECH):
                            nc.vector.tensor_tensor(
                                out=os_sb[:, s % 2, e * 512 : (e + 1) * 512],
                                in0=pss[m][e][:, :],
                                in1=bb_sb[:, e * 512 : (e + 1) * 512],
                                op=ALU.add,
                            )
                        if s % 2 == 1:
                            q = s // 2
                            if q == NOP - 1:
                                # final pair: split by partition halves across
                                # both free queues to shorten the tail
                                nc.sync.dma_start(
                                    out=out[q, 0:64, :, :],
                                    in_=os_sb.bitcast(dt.float32)[0:64, :, :],
                                )
                                nc.gpsimd.dma_start(
                                    out=out[q, 64:128, :, :],
                                    in_=os_sb.bitcast(dt.float32)[64:128, :, :],
                                )
                            else:
                                nc.sync.dma_start(
                                    out=out[q, :, :, :],
                                    in_=os_sb.bitcast(dt.float32),
                                )

            ys = []
            for c in range(NCH):
                ys.append(emit_conv(c))
                if c >= 1:
                    emit_pe(c - 1, ys[c - 1])
            emit_pe(NCH - 1, ys[NCH - 1])

    nc.compile()
    _BUILT["nc"] = nc
    return nc


def _shard_inputs(x, conv_w, conv_b, W_proj, b_proj):
    bf16 = ml_dtypes.bfloat16
    x = np.ascontiguousarray(x, dtype=np.float32)
    # W pairs: pair p = channel tiles (2p, 2p+1) interleaved on partitions
    wt_np = (
        np.ascontiguousarray(W_proj.T, dtype=np.float32)
        .astype(bf16)
        .reshape(NWP, 2, 128, D)
        .transpose(0, 2, 1, 3)
        .copy()
        .view(np.float32)
    )
    cw_np = np.ascontiguousarray(
        conv_w.reshape(KT, 128, KW).transpose(1, 0, 2).reshape(128, KT * KW),
        dtype=np.float32,
    )
    cb_np = np.ascontiguousarray(conv_b.reshape(KT, 128).T, dtype=np.float32)
    bp_np = np.ascontiguousarray(b_proj.reshape(1, D), dtype=np.float32)

    per_batch = S // T
    in_maps = []
    for c in range(NCORES):
        b = c // per_batch
        s0 = (c % per_batch) * T
        xp = np.zeros((T + 3, D), dtype=np.float32)
        xp[3:] = x[b, s0 : s0 + T]
        if s0 > 0:
            xp[:3] = x[b, s0 - 3 : s0]
        xTc = xp.T  # [D, T+3]
        # [NCH, D, CW+3] sliding chunks -> halves [NCH*2, 128, JH*(CW+3)]
        chunks = np.stack([xTc[:, i * CW : i * CW + CW + 3] for i in range(NCH)])
        chunks = chunks.reshape(NCH, 2, JH, 128, CW + 3)
        halves = np.ascontiguousarray(chunks.transpose(0, 1, 3, 2, 4)).reshape(
            NCH * 2, 128, XW
        )
        in_maps.append(
            {
                "xs_t": halves.astype(bf16).view(np.float32),
                "wt": wt_np,
                "cw": cw_np,
                "cb": cb_np,
                "bp": bp_np,
            }
        )
    return in_maps


def run_sharded(x, conv_w, conv_b, W_proj, b_proj, trace=False):
    """Run across the 8 cores; returns (full_out [B,S,D], BassKernelResults)."""
    from concourse.bass_utils import run_bass_kernel_spmd

    nc = _build_program()
    in_maps = _shard_inputs(x, conv_w, conv_b, W_proj, b_proj)
    try:
        res = run_bass_kernel_spmd(nc, in_maps, list(range(NCORES)), trace=trace)
    except Exception:
        # transient device wedges (NRT_EXEC_UNIT_UNRECOVERABLE) clear on retry
        res = run_bass_kernel_spmd(nc, in_maps, list(range(NCORES)), trace=trace)
    full = np.empty((B, S, D), dtype=np.float32)
    per_batch = S // T
    for c in range(NCORES):
        b = c // per_batch
        s0 = (c % per_batch) * T
        o = np.ascontiguousarray(res.results[c]["out"])  # [NOP, 128, 2, D//2] f32
        o = (
            o.view(ml_dtypes.bfloat16)
            .reshape(NOP, 128, 2, D)
            .transpose(0, 2, 1, 3)
            .reshape(T, D)
            .astype(np.float32)
        )
        full[b, s0 : s0 + T] = o
    return full, res


def kernel(x, conv_w, conv_b, W_proj, b_proj):
    full, _ = run_sharded(x, conv_w, conv_b, W_proj, b_proj, trace=False)
    return full


# revision 9
# speedup vs baseline: 1.1593x; 1.1242x over previous
"""Trainium2 Bass kernel for nn_Expert_13082470383822.

y = silu(depthwise_causal_conv1d(x, conv_w, K=4) + conv_b);  out = y @ W_proj.T + b_proj
x [4, 4096, 2048] fp32. Data-parallel over the 16384 (batch*seq) tokens across
8 NeuronCores (2048 tokens/core + 3-token halo).

All-bf16 dataflow (x, W, y, out in bf16; PSUM accum fp32), every bulk transfer
fp32-packed (the DMA engines are element-rate limited) and ~1MB-sized (each of
the three DMA rings is a serial FIFO with ~5us per transfer, sharing ~350GB/s
of HBM).

Ring schedule, matched to the PE's in-order W consumption during the chase:
  SP  : x piece0, W pairs 0,3,6, then the output strip-pairs
  ACT : x piece1, W pairs 2,5, then x pieces 2-7 (issues injected into the
        conv stream so the ACT engine never blocks on them)
  GPS : conv consts, W pairs 1,4,7, bias broadcast, final out half
DMA issues wait on semaphore reuse on the ISSUING engine, so compute engines
only carry issues whose semaphores are provably fresh or long-retired.

Work is organized in 256-token pieces (2 matmul strips): conv for piece p+1 is
emitted before piece p's matmuls, sized so the DVE's per-piece load (3 taps x
16 channel tiles + 8 PSUM drains ~ 24.6us) fits inside the PE's 28us per
piece. Piece 0 interleaves its two strips per channel tile (8 matmuls per W
tile) to chase the W stream; later pieces run strip-by-strip with 4-bank PSUM
ping-pong so drains overlap the next strip's matmuls. b_proj is added during
the PSUM->SBUF drain on DVE; outputs leave as fp32-packed bf16 strip-pairs.
"""

import sys

if "/opt/trn_rl_repo" not in sys.path:
    sys.path.insert(0, "/opt/trn_rl_repo")

import numpy as np
import ml_dtypes

B, S, D, KW = 4, 4096, 2048, 4
NCORES = 8
T = (B * S) // NCORES  # tokens per core = 2048
KT = D // 128  # 16 channel tiles
ECH = D // 512  # 4 e-chunks
CW = 256  # conv piece width (tokens)
MS = 128  # matmul strip width (tokens)
NP = T // CW  # 8 conv pieces
XW = KT * (CW + 3)  # flat bf16 elements per x piece row (4144)
NWP = KT // 2  # 8 W pair tiles

_BUILT = {}


def _build_program():
    if "nc" in _BUILT:
        return _BUILT["nc"]

    import concourse.tile as tile
    from concourse import bacc, mybir

    dt = mybir.dt
    AF = mybir.ActivationFunctionType
    ALU = mybir.AluOpType

    nc = bacc.Bacc("TRN2", target_bir_lowering=False, debug=False)
    xs_d = nc.declare_dram_parameter(
        "xs_t", [NP, 128, XW // 2], dt.float32, isOutput=False
    )
    wt = nc.declare_dram_parameter(
        "wt", [NWP, 128, 2, D // 2], dt.float32, isOutput=False
    )
    cw = nc.declare_dram_parameter("cw", [128, KT * KW], dt.float32, isOutput=False)
    cb = nc.declare_dram_parameter("cb", [128, KT], dt.float32, isOutput=False)
    bp = nc.declare_dram_parameter("bp", [1, D], dt.float32, isOutput=False)
    out = nc.declare_dram_parameter(
        "out", [NP, 128, 2, D // 2], dt.float32, isOutput=True
    )

    with tile.TileContext(nc) as tc:
        with (
            tc.tile_pool(name="consts", bufs=1) as cpool,
            tc.tile_pool(name="wpool", bufs=1) as wpool,
            tc.tile_pool(name="xpool", bufs=4) as xpool,
            tc.tile_pool(name="ypool", bufs=3) as ypool,
            tc.tile_pool(name="apool", bufs=4) as apool,
            tc.tile_pool(name="opool", bufs=2) as opool,
            tc.tile_pool(name="pspool", bufs=8, space="PSUM") as pspool,
        ):
            w_sb = [
                wpool.tile([128, 2, D], dt.bfloat16, name=f"w{p}") for p in range(NWP)
            ]

            def w_dma(eng, p):
                eng.dma_start(out=w_sb[p].bitcast(dt.float32), in_=wt[p, :, :, :])

            def x_dma(eng, p, store):
                xt = xpool.tile([128, XW], dt.bfloat16, name="xs", tag="xs")
                eng.dma_start(out=xt.bitcast(dt.float32), in_=xs_d[p, :, :])
                store.append(xt)

            # x pieces 0,1 lead the two HW rings; W pairs follow in
            # consumption-order round-robin across all three rings
            xtiles = []
            x_dma(nc.sync, 0, xtiles)
            x_dma(nc.scalar, 1, xtiles)
            w_dma(nc.sync, 0)
            w_dma(nc.scalar, 2)
            w_dma(nc.scalar, 5)
            w_dma(nc.sync, 3)
            w_dma(nc.sync, 6)

            # warm the ACT function table
            dum = cpool.tile([1, 1], dt.float32, name="dum")
            nc.gpsimd.memset(dum[:, :], 0.0)
            nc.scalar.activation(dum[:, :], dum[:, :], AF.Silu, bias=0.0)

            cw_sb = cpool.tile([128, KT * KW], dt.float32, name="cw_sb")
            nc.gpsimd.dma_start(out=cw_sb[:, :], in_=cw[:, :])
            cb_sb = cpool.tile([128, KT], dt.float32, name="cb_sb")
            nc.gpsimd.dma_start(out=cb_sb[:, :], in_=cb[:, :])
            w_dma(nc.gpsimd, 1)
            w_dma(nc.gpsimd, 4)
            w_dma(nc.gpsimd, 7)
            bb_sb = cpool.tile([128, D], dt.float32, name="bb_sb")
            nc.gpsimd.dma_start(out=bb_sb[:, :], in_=bp[:, :].broadcast_to([128, D]))

            def w_ap(j, e):
                return w_sb[j // 2][:, j % 2, e * 512 : (e + 1) * 512]

            def emit_conv(p):
                pieces = xtiles[p]
                y = ypool.tile([128, KT, CW], dt.bfloat16, name="ys", tag="ys")
                accs = [None] * KT
                accs[0] = apool.tile([128, CW], dt.bfloat16, name="acc", tag="acc")
                nc.scalar.activation(
                    accs[0][:, :],
                    pieces[:, 0:CW],
                    AF.Copy,
                    bias=0.0,
                    scale=cw_sb[:, 0:1],
                )
                for j in range(KT):
                    base = j * (CW + 3)
                    for k in range(1, KW):
                        nc.vector.scalar_tensor_tensor(
                            accs[j][:, :],
                            pieces[:, base + k : base + k + CW],
                            cw_sb[:, j * KW + k : j * KW + k + 1],
                            accs[j][:, :],
                            ALU.mult,
                            ALU.add,
                        )
                    if j + 1 < KT:
                        accs[j + 1] = apool.tile(
                            [128, CW], dt.bfloat16, name="acc", tag="acc"
                        )
                        nb = (j + 1) * (CW + 3)
                        nc.scalar.activation(
                            accs[j + 1][:, :],
                            pieces[:, nb : nb + CW],
                            AF.Copy,
                            bias=0.0,
                            scale=cw_sb[:, (j + 1) * KW : (j + 1) * KW + 1],
                        )
                    nc.scalar.activation(
                        y[:, j, :],
                        accs[j][:, :],
                        AF.Silu,
                        bias=cb_sb[:, j : j + 1],
                    )
                    if j == 2 and len(xtiles) < NP:
                        # stage the next x piece on the ACT ring mid-conv; the
                        # issue's semaphore is many transfers old, so the ACT
                        # engine never waits here
                        x_dma(nc.scalar, len(xtiles), xtiles)
                return y

            def drain(p, m, pss, os_sb):
                for e in range(ECH):
                    nc.vector.tensor_tensor(
                        out=os_sb[:, m, e * 512 : (e + 1) * 512],
                        in0=pss[e][:, :],
                        in1=bb_sb[:, e * 512 : (e + 1) * 512],
                        op=ALU.add,
                    )
                if m == 1:
                    if p == NP - 1:
                        # final pair: split by partition halves across both
                        # free rings to shorten the tail
                        nc.sync.dma_start(
                            out=out[p, 0:64, :, :],
                            in_=os_sb.bitcast(dt.float32)[0:64, :, :],
                        )
                        nc.gpsimd.dma_start(
                            out=out[p, 64:128, :, :],
                            in_=os_sb.bitcast(dt.float32)[64:128, :, :],
                        )
                    else:
                        nc.sync.dma_start(
                            out=out[p, :, :, :], in_=os_sb.bitcast(dt.float32)
                        )

            def emit_pe(p, y):
                os_sb = opool.tile([128, 2, D], dt.bfloat16, name="os", tag="os")
                if p == 0:
                    # chase the W stream: both strips per channel tile
                    pss = {
                        m: [
                            pspool.tile([128, 512], dt.float32, name="ps", tag="ps")
                            for _ in range(ECH)
                        ]
                        for m in (0, 1)
                    }
                    for j in range(KT):
                        for m in (0, 1):
                            for e in range(ECH):
                                nc.tensor.matmul(
                                    pss[m][e][:, :],
                                    y[:, j, m * MS : (m + 1) * MS],
                                    w_ap(j, e),
                                    start=(j == 0),
                                    stop=(j == KT - 1),
                                )
                    for m in (0, 1):
                        drain(p, m, pss[m], os_sb)
                else:
                    # strip-sequential with 4-bank ping-pong: strip m's drains
                    # overlap strip m+1's matmuls
                    for m in (0, 1):
                        pss = [
                            pspool.tile([128, 512], dt.float32, name="ps", tag="ps")
                            for _ in range(ECH)
                        ]
                        for j in range(KT):
                            for e in range(ECH):
                                nc.tensor.matmul(
                                    pss[e][:, :],
                                    y[:, j, m * MS : (m + 1) * MS],
                                    w_ap(j, e),
                                    start=(j == 0),
                                    stop=(j == KT - 1),
                                )
                        drain(p, m, pss, os_sb)

            ys = []
            for p in range(NP):
                ys.append(emit_conv(p))
                if p >= 1:
                    emit_pe(p - 1, ys[p - 1])
            emit_pe(NP - 1, ys[NP - 1])

    nc.compile()
    _BUILT["nc"] = nc
    return nc


def _shard_inputs(x, conv_w, conv_b, W_proj, b_proj):
    bf16 = ml_dtypes.bfloat16
    x = np.ascontiguousarray(x, dtype=np.float32)
    wt_np = (
        np.ascontiguousarray(W_proj.T, dtype=np.float32)
        .astype(bf16)
        .reshape(NWP, 2, 128, D)
        .transpose(0, 2, 1, 3)
        .copy()
        .view(np.float32)
    )
    cw_np = np.ascontiguousarray(
        conv_w.reshape(KT, 128, KW).transpose(1, 0, 2).reshape(128, KT * KW),
        dtype=np.float32,
    )
    cb_np = np.ascontiguousarray(conv_b.reshape(KT, 128).T, dtype=np.float32)
    bp_np = np.ascontiguousarray(b_proj.reshape(1, D), dtype=np.float32)

    per_batch = S // T
    in_maps = []
    for c in range(NCORES):
        b = c // per_batch
        s0 = (c % per_batch) * T
        xp = np.zeros((T + 3, D), dtype=np.float32)
        xp[3:] = x[b, s0 : s0 + T]
        if s0 > 0:
            xp[:3] = x[b, s0 - 3 : s0]
        xTc = xp.T  # [D, T+3]
        pieces = np.stack([xTc[:, i * CW : i * CW + CW + 3] for i in range(NP)])
        pieces = pieces.reshape(NP, KT, 128, CW + 3)
        pieces = np.ascontiguousarray(pieces.transpose(0, 2, 1, 3)).reshape(
            NP, 128, XW
        )
        in_maps.append(
            {
                "xs_t": pieces.astype(bf16).view(np.float32),
                "wt": wt_np,
                "cw": cw_np,
                "cb": cb_np,
                "bp": bp_np,
            }
        )
    return in_maps


def run_sharded(x, conv_w, conv_b, W_proj, b_proj, trace=False):
    """Run across the 8 cores; returns (full_out [B,S,D], BassKernelResults)."""
    from concourse.bass_utils import run_bass_kernel_spmd

    nc = _build_program()
    in_maps = _shard_inputs(x, conv_w, conv_b, W_proj, b_proj)
    try:
        res = run_bass_kernel_spmd(nc, in_maps, list(range(NCORES)), trace=trace)
    except Exception:
        # transient device wedges (NRT_EXEC_UNIT_UNRECOVERABLE) clear on retry
        res = run_bass_kernel_spmd(nc, in_maps, list(range(NCORES)), trace=trace)
    full = np.empty((B, S, D), dtype=np.float32)
    per_batch = S // T
    for c in range(NCORES):
        b = c // per_batch
        s0 = (c % per_batch) * T
        o = np.ascontiguousarray(res.results[c]["out"])  # [NP, 128, 2, D//2] f32
        o = (
            o.view(ml_dtypes.bfloat16)
            .reshape(NP, 128, 2, D)
            .transpose(0, 2, 1, 3)
            .reshape(T, D)
            .astype(np.float32)
        )
        full[b, s0 : s0 + T] = o
    return full, res


def kernel(x, conv_w, conv_b, W_proj, b_proj):
    full, _ = run_sharded(x, conv_w, conv_b, W_proj, b_proj, trace=False)
    return full
